# revision 72
# baseline (speedup 1.0000x reference)
"""Trainium2 Bass kernel v3 for a GQA attention layer (B=2, S=2048, D=4096,
32 Q heads / 8 KV heads, rotary, causal additive mask), SPMD on 8 cores.

Causal load balance with a UNIFORM per-core program: each core owns 8
query tiles of 64 tokens, tile m (m=0..7) = global 64-tile j = 3+4m-q
(q = core%4, batch = core//4).  With this zig-zag striding every core's
causal attention footprint pads to the same per-chunk suffix structure:
key chunk g (128 keys) is consumed by local blocks m >= MG[g], so the
instruction stream is identical on all cores and only the DATA (token
permutation, mask tiles) differs.  Attention per core: 36 (128q x 128k)
tile-ops vs 64 for the naive full-key sweep.

All 16-bit data is fp16 (not bf16): same PE throughput, 8x lower
rel-err (~1e-3), with the additive -inf mask clamped to -60000 so it
stays fp16-representable (exp(SCALE*(s-60000)) == 0).

Scores are computed transposed (S^T = K_chunk^T.T @ Q^T suffix), the
mask is accumulated ON THE PE via an identity-stationary matmul, exp
runs on ACT reading PSUM directly (fp16 out), A*V accumulates on the
PE into per-head [128,512] PSUM tiles, and the softmax denominator is
folded on the DVE into a per-head fp16 accumulator (query-aligned
suffix adds) so ONE 512-col all-ones matmul per head produces Z
(saves ~57k PE columns vs per-pair Z matmuls).

K/V are projected for local tokens only and shared within each batch's
4 cores via four PER-KV-HEAD-PAIR AllGathers: pairs 0/1 fire as soon
as their data lands (late V phase), pairs 2/3 are emitted inside the
attention loop so their link traffic and rendezvous land in DMA-quiet
windows (the projection phase is DMA-descriptor-line-bound, and an
active/waiting collective starves the DMA rings).  Weights are
host-packed in d-QUADS ([128, 4w] tiles -> 3-4KB DMA lines, half the
descriptor-line count) and xt is packed as one [128, 32*512] tile
loaded in 4KB-line chunks.  wo produces each core's disjoint 512
output rows; the host scatters them back.
"""

import os
import sys
from contextlib import ExitStack
from dataclasses import dataclass

import numpy as np

if os.path.isdir("/opt/trn_rl_repo") and "/opt/trn_rl_repo" not in sys.path:
    sys.path.insert(0, "/opt/trn_rl_repo")

import ml_dtypes

import concourse.bass as bass  # noqa: F401  (kept for parity with utils)
import concourse.mybir as mybir
import concourse.tile as tile
from concourse import bacc
from concourse.bass_utils import run_bass_kernel_spmd

BF16 = mybir.dt.bfloat16
F16 = mybir.dt.float16
F32 = mybir.dt.float32
NPBF16 = ml_dtypes.bfloat16
NPF16 = np.float16
P = 128
# fp16 stand-in for -inf mask (pre-scale): exp(SCALE*(s + MASK_NEG)) == 0
MASK_NEG = -60000.0


@dataclass(frozen=True)
class Cfg:
    S: int = 2048      # full sequence
    D: int = 4096      # model dim
    NH: int = 32       # query heads
    NKV: int = 8       # kv heads
    HD: int = 128      # head dim (must equal P)

    @property
    def T(self):
        return self.S // 4          # tokens per core (512)

    @property
    def DT(self):
        return self.D // P          # 32

    @property
    def NCH(self):
        return self.S // P          # 16 key chunks

    @property
    def NREP(self):
        return self.NH // self.NKV


FULL = Cfg()

# --- static causal structure (64-token query blocks, zig-zag striding) ---
# local block m (0..7) on core q = global 64-tile j = 3 + 4m - q.
# chunk g is live for local blocks m >= MG[g]; width WG[g] = 64*(8-MG[g]).
MG = [max(0, -(-(2 * g - 3) // 4)) for g in range(16)]
WG = [64 * (8 - m) for m in MG]
assert sum(WG) == 4608

# bins: chunks packed into [128, <=1024] score tiles; (g, col_off) pairs.
# No chunk's segment may cross a 512-col (PSUM bank) boundary.  Within a
# shared bank the SMALL chunk goes first so both chunks' mask columns
# ([off, off+64)) are adjacent and one identity matmul adds both masks.
BINS = [
    [(0, 0), (1, 512)],
    [(14, 0), (2, 64), (15, 512), (3, 576)],
    [(12, 0), (4, 128), (13, 512), (5, 640)],
    [(10, 0), (6, 192), (11, 512), (7, 704)],
    [(8, 0), (9, 256)],
]
BIN_W = [max(off + WG[g] for g, off in b) for b in BINS]
for b in BINS:
    for g, off in b:
        assert off // 512 == (off + WG[g] - 1) // 512, (g, off)
assert sorted(g for b in BINS for g, _ in b) == list(range(16))

# mask matmuls: one per (bin, bank) when the bank's mask columns are
# adjacent, else one per chunk.  Entries: (dst_off, [gs], src_col) with
# maskp columns laid out per-MM at src_col (64 cols per chunk).
MASK_MMS = []          # per bin: list of (dst_off, gs, src_col)
_slot = 0
for b in BINS:
    mms = []
    for bank in (0, 1):
        segs = [(g, off) for g, off in b if off // 512 == bank]
        segs.sort(key=lambda t: t[1])
        while segs:
            g0, o0 = segs.pop(0)
            gs = [g0]
            while segs and segs[0][1] == o0 + 64 * len(gs):
                gs.append(segs.pop(0)[0])
            mms.append((o0, gs, _slot))
            _slot += 64 * len(gs)
    MASK_MMS.append(mms)
assert _slot == 16 * 64
MASK_SRC = {}          # g -> maskp column
for mms in MASK_MMS:
    for o0, gs, sc in mms:
        for i, g in enumerate(gs):
            MASK_SRC[g] = sc + 64 * i

# chunk pairs (2g, 2g+1) share MG (same live query suffix) and sit in the
# same bin -> their exp tiles are summed on the DVE so ONE Z matmul per
# pair feeds the softmax denominator.
BIN_PAIRS = []
for b in BINS:
    offs = dict(b)
    pairs = []
    gs = sorted(offs)
    for g in gs:
        if g % 2 == 0:
            assert g + 1 in offs and MG[g] == MG[g + 1], b
            pairs.append((g, offs[g], offs[g + 1]))
    assert 2 * len(pairs) == len(b)
    BIN_PAIRS.append(pairs)
ZP_FIRST = BIN_PAIRS[0][0][0]       # pair lead chunk initializing zb
ZP_LAST = BIN_PAIRS[-1][-1][0]

AV_FIRST = BINS[0][0][0]           # chunk 0 (full-width, inits the bank)
AV_LAST = BINS[-1][-1][0]          # last emitted chunk
assert WG[AV_FIRST] == 512


def local_tokens(q, cfg: Cfg = FULL):
    """Global token indices (within the batch) owned by core-quarter q,
    in local order (8 blocks of 64)."""
    toks = []
    for m in range(8):
        j = 3 + 4 * m - q
        toks.extend(range(64 * j, 64 * j + 64))
    return np.array(toks)


def groups_of3(n):
    return [list(range(k, min(k + 3, n))) for k in range(0, n, 3)]


def pack_colgroups_dquads(wT, groups, DT):
    """wT: [D, E] contraction-major.  Flat layout: [group][dquad][128, 4w]
    contiguous blocks (d..d+3 side by side) -> 3-4KB DMA lines."""
    blocks = []
    for grp in groups:
        c0, w = grp[0] * P, len(grp) * P
        for dq in range(DT // 4):
            parts = [wT[(4 * dq + i) * P:(4 * dq + i + 1) * P, c0:c0 + w]
                     for i in range(4)]
            blocks.append(np.ascontiguousarray(
                np.concatenate(parts, axis=1)).reshape(-1))
    return np.concatenate(blocks)


def classify(mask, cfg: Cfg = FULL):
    """Verify the mask matches the causal zig-zag structure this kernel
    bakes in.  Returns a signature (currently fixed)."""
    mask = np.asarray(mask, np.float32)
    for q in range(4):
        toks = local_tokens(q, cfg)
        for g in range(cfg.NCH):
            for m in range(8):
                sub = mask[toks[64 * m:64 * m + 64], g * P:(g + 1) * P]
                full = bool((sub <= -1e8).all())
                zero = bool(not sub.any())
                if m < MG[g]:
                    assert full, (q, g, m, "expected fully-masked (skipped)")
                elif m > MG[g]:
                    assert zero, (q, g, m, "expected fully-visible")
    return "causal-v2"


def build_nc(cfg: Cfg):
    D, NH, NKV, HD = cfg.D, cfg.NH, cfg.NKV, cfg.HD
    T, DT, NCH = cfg.T, cfg.DT, cfg.NCH
    KVW = NKV * HD                      # 1024
    NDO = D // 512                      # 8 wo output column groups
    SCALE = float(np.float32(1.0) / np.float32(np.sqrt(np.float32(HD))))
    VBASE = KVW * T                     # V region offset in kvin (elements)
    KVIN_E = 2 * KVW * T                # elements per core's kv contribution

    kgroups = groups_of3(NKV)
    qgroups = groups_of3(NH)
    vgroups = [(eh, tss) for eh in range(2) for tss in groups_of3(T // P)]

    nc = bacc.Bacc("TRN2", target_bir_lowering=False, debug=False, num_devices=8)

    xt_d = nc.dram_tensor("xt", [P, (D // P) * T], F16, kind="ExternalInput")
    wqp_d = nc.dram_tensor("wqp", [D * NH * HD], F16, kind="ExternalInput")
    wkp_d = nc.dram_tensor("wkp", [D * KVW], F16, kind="ExternalInput")
    wvp_d = nc.dram_tensor("wvp", [D * KVW], F16, kind="ExternalInput")
    wop_d = nc.dram_tensor("wop", [NH * HD * D], F16, kind="ExternalInput")
    cost_d = nc.dram_tensor("cost", [HD, T], F32, kind="ExternalInput")
    sint_d = nc.dram_tensor("sint", [HD, T], F32, kind="ExternalInput")
    maskp_d = nc.dram_tensor("maskp", [P, NCH * 64], F16, kind="ExternalInput")
    swap_d = nc.dram_tensor("swapm", [P, P], F16, kind="ExternalInput")
    ones_d = nc.dram_tensor("onesmat", [P, P], F16, kind="ExternalInput")
    ident_d = nc.dram_tensor("identm", [P, P], F16, kind="ExternalInput")
    out_d = nc.dram_tensor("out", [T, D], F32, kind="ExternalOutput")

    def grp_offsets(groups):
        offs, off = [], 0
        for grp in groups:
            offs.append(off)
            off += DT * P * len(grp) * P
        return offs

    qoffs = grp_offsets(qgroups)
    koffs = grp_offsets(kgroups)
    wqp, wkp, wvp, wop = wqp_d.ap(), wkp_d.ap(), wvp_d.ap(), wop_d.ap()

    with tile.TileContext(nc) as tc, ExitStack() as ctx:
        persist = ctx.enter_context(tc.tile_pool(name="persist", bufs=1))
        wpool = ctx.enter_context(tc.tile_pool(name="wpool", bufs=3))
        dramp = ctx.enter_context(tc.tile_pool(name="dramp", bufs=1, space="DRAM"))

        # ---- constants (scalar queue) ----
        swap_sb = persist.tile([P, P], F16, name="swap_sb")
        nc.scalar.dma_start(swap_sb[:], swap_d.ap()[:])
        ones_sb = persist.tile([P, P], F16, name="ones_sb")
        nc.scalar.dma_start(ones_sb[:], ones_d.ap()[:])
        ident_sb = persist.tile([P, P], F16, name="ident_sb")
        nc.scalar.dma_start(ident_sb[:], ident_d.ap()[:])
        cost_sb = persist.tile([HD, T], F32, name="cost_sb")
        nc.scalar.dma_start(cost_sb[:], cost_d.ap()[:])
        sint_sb = persist.tile([HD, T], F32, name="sint_sb")
        nc.scalar.dma_start(sint_sb[:], sint_d.ap()[:])
        maskp_sb = persist.tile([P, NCH * 64], F16, name="maskp_sb")
        nc.scalar.dma_start(maskp_sb[:], maskp_d.ap()[:])

        # kvin grouped by kv-head PAIR pi (kvh 2pi, 2pi+1): rows
        # [pi*4HD, +HD) = K^T of 2pi; [+HD, +2HD) = K^T of 2pi+1;
        # [+2HD, +4HD) = V blocks of both heads ((kvh%2)*HD*T + m*64*HD).
        # One small AllGather per pair, fired late (DMA-quiet windows).
        PRB = 4 * HD                       # rows per pair block
        kvin = dramp.tile([4 * PRB, T], F16, name="kvin")
        kvoutp = [dramp.tile([4 * PRB, T], F16, name=f"kvoutp{pi}")
                  for pi in range(4)]
        kvin_flat = kvin[:].rearrange("a b -> (a b)")
        kvoutp_flats = [t[:].rearrange("a b -> (a b)") for t in kvoutp]

        def emit_gather(pi):
            nc.gpsimd.collective_compute(
                "AllGather",
                mybir.AluOpType.bypass,
                replica_groups=[[0, 1, 2, 3], [4, 5, 6, 7]],
                ins=[kvin[pi * PRB:(pi + 1) * PRB, :].opt()],
                outs=[kvoutp[pi][:].opt()],
            )

        qt = [persist.tile([P, T], F16, name=f"qt_{h}") for h in range(NH)]
        kvp = ctx.enter_context(tc.tile_pool(name="kvp", bufs=1))
        att = [None] * NH          # allocated after xtp frees SBUF
        kvtiles = {}

        def emit_repack(kvh, eng):
            ktf = kvp.tile([P, 16 * P], F16, tag="kt", bufs=3,
                           name=f"ktf_{kvh}")
            vtf = kvp.tile([P, 16 * P], F16, tag="vt", bufs=3,
                           name=f"vtf_{kvh}")
            kvtiles[kvh] = (ktf, vtf)
            pi = kvh // 2
            for r in range(4):
                # K^T: [128, m(8), 64] -> global cols 64*(3+4m-r)+u
                src = kvoutp[pi][r * PRB + (kvh % 2) * HD:
                                 r * PRB + (kvh % 2) * HD + HD, :] \
                    .rearrange("p (m o u) -> p m o u", m=8, o=1, u=64)
                dst = ktf[:].rearrange("p (m f u) -> p m f u", f=4, u=64) \
                    [:, :, 3 - r:4 - r, :]
                eng.dma_start(dst, src)
                # V: blocks [64 tok, 128 hd] -> chunk halves
                base = (r * PRB + 2 * HD + (kvh % 2) * HD) * T
                srcv = kvoutp_flats[pi][base:base + 8 * 64 * HD] \
                    .rearrange("(m o p f) -> p m o f", m=8, o=1, p=64, f=HD)
                pp = 64 if r in (0, 2) else 0
                twoidx = 1 if r in (0, 1) else 0
                dstv = vtf[pp:pp + 64, :] \
                    .rearrange("p (m t f) -> p m t f", t=2, f=HD) \
                    [:, :, twoidx:twoidx + 1, :]
                eng.dma_start(dstv, srcv)

        with tc.tile_pool(name="xtp", bufs=1) as xtp, \
             tc.tile_pool(name="rot", bufs=2) as rot, \
             tc.tile_pool(name="psP", bufs=1, space="PSUM") as psP:

            def rotary(raw_ps, dst_f16, nm):
                raw = rot.tile([P, T], F16, tag="raw", bufs=6, name=f"raw_{nm}")
                nc.scalar.copy(raw[:], raw_ps[:])
                sw_ps = psP.tile([P, T], F32, tag="swp", bufs=2, name=f"swp_{nm}")
                nc.tensor.matmul(sw_ps[:], swap_sb[:], raw[:], start=True, stop=True)
                t1 = rot.tile([P, T], F32, tag="t1", bufs=4, name=f"t1_{nm}")
                nc.vector.tensor_mul(t1[:], raw[:], cost_sb[:])
                t2 = rot.tile([P, T], F32, tag="t2", bufs=3, name=f"t2_{nm}")
                nc.vector.tensor_mul(t2[:], sw_ps[:], sint_sb[:])
                nc.vector.tensor_add(dst_f16[:], t1[:], t2[:])

            xt_all = xtp.tile([P, DT * T], F16, name="xt_all")
            xt_sb = [xt_all[:, d * T:(d + 1) * T] for d in range(DT)]
            xt_loaded = [False] * (DT // 4)

            def load_xt(d):
                dq = d // 4
                if not xt_loaded[dq]:
                    # one 4-d-tile chunk: 128 lines of 4KB
                    nc.sync.dma_start(xt_all[:, dq * 4 * T:(dq + 1) * 4 * T],
                                      xt_d.ap()[:, dq * 4 * T:(dq + 1) * 4 * T])
                    xt_loaded[dq] = True

            # ---- K^T projection (local tokens) + rotary ----
            ktloc = [persist.tile([P, T], F16, name=f"ktloc_{kvh}")
                     for kvh in range(NKV)]
            for gi, grp in enumerate(kgroups):
                w = len(grp) * P
                kps = [psP.tile([P, T], F32, tag=f"pj{j}", bufs=2,
                                name=f"kps_{gi}_{j}") for j in range(len(grp))]
                for dq in range(DT // 4):
                    wrow = wpool.tile([P, 4 * 3 * P], F16, tag="wkv", bufs=5,
                                      name=f"wk_{gi}_{dq}")
                    off = koffs[gi] + dq * P * 4 * w
                    wsrc = wkp[off:off + P * 4 * w].rearrange("(p f) -> p f", p=P)
                    if gi == 0 and dq == 0:
                        # split the very first weight load so the opening
                        # matmul only waits on part of the bytes
                        nc.sync.dma_start(wrow[:, :w], wsrc[:, :w])
                        nc.sync.dma_start(wrow[:, w:4 * w], wsrc[:, w:])
                    else:
                        nc.sync.dma_start(wrow[:, :4 * w], wsrc)
                    load_xt(4 * dq)
                    for quarter in range(4):
                        d = 4 * dq + quarter
                        for j in range(len(grp)):
                            nc.tensor.matmul(
                                kps[j][:],
                                wrow[:, quarter * w + j * HD:
                                     quarter * w + (j + 1) * HD],
                                xt_sb[d][:],
                                start=(dq == 0 and quarter == 0),
                                stop=(dq == DT // 4 - 1 and quarter == 3))
                for j, kvh in enumerate(grp):
                    rotary(kps[j], ktloc[kvh], f"k{kvh}")
                    nc.scalar.dma_start(
                        kvin[(kvh // 2) * PRB + (kvh % 2) * HD:
                             (kvh // 2) * PRB + (kvh % 2) * HD + HD, :],
                        ktloc[kvh][:])

            # ---- V projection (local tokens), [token, feature] layout ----
            vtloc = [xtp.tile([P, KVW], F16, name=f"vtloc_{ts}")
                     for ts in range(T // P)]
            for gi, (eh, tss) in enumerate(vgroups):
                vps = [psP.tile([P, 512], F32, tag=f"pj{j}", bufs=2,
                                name=f"vps_{gi}_{j}") for j in range(len(tss))]
                for dq in range(DT // 4):
                    wrow = wpool.tile([P, 4 * 512], F16, tag="wvr", bufs=4,
                                      name=f"wv_{gi}_{dq}")
                    off = (eh * (DT // 4) + dq) * P * 2048
                    wsrc = wvp[off:off + P * 2048].rearrange("(p f) -> p f", p=P)
                    nc.sync.dma_start(wrow[:], wsrc)
                    for quarter in range(4):
                        d = 4 * dq + quarter
                        for j, ts in enumerate(tss):
                            nc.tensor.matmul(
                                vps[j][:], xt_sb[d][:, ts * P:(ts + 1) * P],
                                wrow[:, quarter * 512:(quarter + 1) * 512],
                                start=(dq == 0 and quarter == 0),
                                stop=(dq == DT // 4 - 1 and quarter == 3))
                for j, ts in enumerate(tss):
                    nc.scalar.copy(vtloc[ts][:, eh * 512:(eh + 1) * 512], vps[j][:])
                if tss[-1] == T // P - 1:
                    # this eh half (kv heads 4*eh..4*eh+3) is now complete
                    # across all token tiles -> pack it into the pair blocks
                    # of kvin (the per-pair gathers fire later, in DMA-quiet
                    # windows).
                    for ts in range(T // P):
                        for kvh in range(4 * eh, 4 * eh + 4):
                            # blocks m=2ts and 2ts+1 are contiguous in kvin:
                            # one [128,128] DMA per (ts, kvh), split across
                            # the scalar and sync queues
                            off = ((kvh // 2) * PRB + 2 * HD
                                   + (kvh % 2) * HD) * T + 2 * ts * (64 * HD)
                            eng = nc.scalar if kvh % 2 else nc.sync
                            eng.dma_start(
                                kvin_flat[off:off + P * HD]
                                .rearrange("(p f) -> p f", p=P),
                                vtloc[ts][:, kvh * HD:(kvh + 1) * HD])

            # ---- Q^T projection + rotary ----
            # pair gathers 0/1 fire as soon as their K+V inputs land;
            # 2/3 are held back to the attention phase (DMA-quiet window)
            emit_gather(0)
            emit_gather(1)

            for gi, grp in enumerate(qgroups):
                if gi == 9:
                    emit_repack(0, nc.scalar)
                elif gi == 10:
                    emit_repack(1, nc.scalar)
                w = len(grp) * P
                qps = [psP.tile([P, T], F32, tag=f"pj{j}", bufs=2,
                                name=f"qps_{gi}_{j}") for j in range(len(grp))]
                for dq in range(DT // 4):
                    wrow = wpool.tile([P, 4 * 3 * P], F16, tag="wq", bufs=10,
                                      name=f"wq_{gi}_{dq}")
                    off = qoffs[gi] + dq * P * 4 * w
                    nc.sync.dma_start(
                        wrow[:, :4 * w],
                        wqp[off:off + P * 4 * w].rearrange("(p f) -> p f", p=P))
                    for quarter in range(4):
                        d = 4 * dq + quarter
                        for j in range(len(grp)):
                            nc.tensor.matmul(
                                qps[j][:],
                                wrow[:, quarter * w + j * HD:
                                     quarter * w + (j + 1) * HD],
                                xt_sb[d][:],
                                start=(dq == 0 and quarter == 0),
                                stop=(dq == DT // 4 - 1 and quarter == 3))
                for j, h in enumerate(grp):
                    rotary(qps[j], qt[h], f"q{h}")

        # ---- attention: per kv head, uniform causal zig-zag sweep ----
        attp = ctx.enter_context(tc.tile_pool(name="attp", bufs=1))
        with tc.tile_pool(name="atw", bufs=1) as work, \
             tc.tile_pool(name="psA", bufs=1, space="PSUM") as psA:
            emit_repack(2, nc.gpsimd)
            for kvh in range(NKV):
                if kvh == 0:
                    emit_gather(2)     # needed at attention kvh 4
                elif kvh == 2:
                    emit_gather(3)     # needed at attention kvh 6
                if kvh >= 2 and kvh + 1 < NKV:
                    emit_repack(kvh + 1, nc.gpsimd)
                ktf, vtf = kvtiles.pop(kvh)

                for hh in range(cfg.NREP):
                    h = kvh * cfg.NREP + hh
                    av = psA.tile([P, T], F32, tag="av", bufs=1, name=f"av_{h}")
                    zb = psA.tile([P, T], F32, tag="zb", bufs=1, name=f"zb_{h}")
                    zacc = work.tile([P, T], F16, tag="zacc", bufs=2,
                                     name=f"zacc_{h}")
                    ets = {}

                    def emit_S(b):
                        s = psA.tile([P, 1024], F32, tag="s", bufs=3,
                                     name=f"s_{h}_{b}")
                        # per 512-col bank: scores then mask adds
                        for bank in (0, 1):
                            segs = [(g, off) for g, off in BINS[b]
                                    if off // 512 == bank]
                            if not segs:
                                continue
                            for i, (g, off) in enumerate(segs):
                                nc.tensor.matmul(
                                    s[:, off:off + WG[g]],
                                    ktf[:, g * P:(g + 1) * P],
                                    qt[h][:, 64 * MG[g]:T],
                                    start=(i == 0), stop=False)
                            mms = [mm for mm in MASK_MMS[b]
                                   if mm[0] // 512 == bank]
                            for i, (o0, gs, sc) in enumerate(mms):
                                w = 64 * len(gs)
                                nc.tensor.matmul(
                                    s[:, o0:o0 + w],
                                    ident_sb[:],
                                    maskp_sb[:, sc:sc + w],
                                    start=False, stop=(i == len(mms) - 1))
                        e = work.tile([P, 1024], F16, tag="e", bufs=4,
                                      name=f"e_{h}_{b}")
                        nc.scalar.activation(
                            e[:, :BIN_W[b]], s[:, :BIN_W[b]],
                            mybir.ActivationFunctionType.Exp, scale=SCALE)
                        # fold this bin's chunks into the per-head softmax
                        # denominator accumulator (query-aligned suffixes)
                        if b == 0:
                            nc.vector.tensor_add(
                                zacc[:], e[:, 0:512], e[:, 512:1024])
                        else:
                            for g, off in BINS[b]:
                                q0 = 64 * MG[g]
                                nc.vector.tensor_add(
                                    zacc[:, q0:T], zacc[:, q0:T],
                                    e[:, off:off + WG[g]])
                        ets[b] = e

                    def emit_A(b):
                        e = ets[b]
                        for g, off in BINS[b]:
                            nc.tensor.matmul(
                                av[:, 64 * MG[g]:T], vtf[:, g * P:(g + 1) * P],
                                e[:, off:off + WG[g]],
                                start=(g == AV_FIRST), stop=(g == AV_LAST))

                    for b in range(len(BINS) + 2):
                        if b < len(BINS):
                            emit_S(b)
                        if b >= 2:
                            emit_A(b - 2)

                    # single 512-col softmax-denominator matmul
                    nc.tensor.matmul(zb[:], ones_sb[:], zacc[:],
                                     start=True, stop=True)
                    rzb = work.tile([P, T], F32, tag="rz", bufs=3,
                                    name=f"rz_{h}")
                    nc.vector.reciprocal_approx_fast(out=rzb[:], in_=zb[:])
                    att[h] = attp.tile([P, T], F16, name=f"att_{h}")
                    nc.vector.tensor_mul(att[h][:], av[:], rzb[:])

        # ---- output projection ----
        with tc.tile_pool(name="osbp", bufs=1) as osbp, \
             tc.tile_pool(name="psW", bufs=1, space="PSUM") as psW:
            for douth in range(NDO):
                ops = [psW.tile([P, 512], F32, tag=f"pw{tt}", bufs=2,
                                name=f"ops_{douth}_{tt}")
                       for tt in range(T // P)]
                for eq in range(NH // 4):
                    wrow = wpool.tile([P, 2048], F16, tag="wo", bufs=4,
                                      name=f"wo_{douth}_{eq}")
                    off = (douth * (NH // 4) + eq) * P * 2048
                    nc.sync.dma_start(
                        wrow[:],
                        wop[off:off + P * 2048].rearrange("(p f) -> p f", p=P))
                    for quarter in range(4):
                        e = 4 * eq + quarter
                        for tt in range(T // P):
                            nc.tensor.matmul(
                                ops[tt][:], att[e][:, tt * P:(tt + 1) * P],
                                wrow[:, quarter * 512:(quarter + 1) * 512],
                                start=(eq == 0 and quarter == 0),
                                stop=(eq == NH // 4 - 1 and quarter == 3))
                for tt in range(T // P):
                    osb = osbp.tile([P, 512], F32, tag="osb", bufs=4,
                                    name=f"osb_{douth}_{tt}")
                    nc.scalar.copy(osb[:], ops[tt][:])
                    nc.scalar.dma_start(
                        out_d.ap()[tt * P:(tt + 1) * P,
                                   douth * 512:(douth + 1) * 512],
                        osb[:])

    nc.compile()
    return nc


def make_in_maps(x, freqs_cis, mask, wq, wk, wv, wo, cfg: Cfg = FULL):
    D, HD, DT, NCH = cfg.D, cfg.HD, cfg.DT, cfg.NCH
    SCALE = np.float32(1.0) / np.float32(np.sqrt(np.float32(HD)))
    x = np.asarray(x, np.float32)
    fc = np.asarray(freqs_cis, np.float32)
    mask = np.asarray(mask, np.float32)
    wqt = np.asarray(wq, np.float32).T.astype(NPF16)   # [D, NH*HD]
    wkt = np.asarray(wk, np.float32).T.astype(NPF16)   # [D, KVW]
    wvt = np.asarray(wv, np.float32).T.astype(NPF16)
    wot = np.asarray(wo, np.float32).T.astype(NPF16)   # [NH*HD, D]

    wqp = pack_colgroups_dquads(wqt, groups_of3(cfg.NH), DT)
    wkp = pack_colgroups_dquads(wkt, groups_of3(cfg.NKV), DT)
    wvp = np.concatenate([
        np.ascontiguousarray(np.concatenate(
            [wvt[(4 * dq + i) * P:(4 * dq + i + 1) * P,
                 eh * 512:(eh + 1) * 512] for i in range(4)],
            axis=1)).reshape(-1)
        for eh in range(2) for dq in range(DT // 4)])
    wop = np.concatenate([
        np.ascontiguousarray(np.concatenate(
            [wot[(4 * eq + i) * P:(4 * eq + i + 1) * P,
                 douth * 512:(douth + 1) * 512] for i in range(4)],
            axis=1)).reshape(-1)
        for douth in range(D // 512) for eq in range(cfg.NH // 4)])

    swapm = np.zeros((P, P), np.float32)
    for i in range(P // 2):
        swapm[2 * i, 2 * i + 1] = 1.0
        swapm[2 * i + 1, 2 * i] = 1.0
    swapm = swapm.astype(NPF16)
    onesmat = np.ones((P, P), NPF16)
    identm = np.eye(P, dtype=np.float32).astype(NPF16)

    in_maps = []
    for c in range(8):
        b, q = c // 4, c % 4
        toks = local_tokens(q, cfg)
        xt = np.ascontiguousarray(
            x[b, toks, :].T.reshape(cfg.DT, P, cfg.T)
            .transpose(1, 0, 2).reshape(P, cfg.DT * cfg.T)).astype(NPF16)
        cost = np.repeat(fc[toks, :, 0].T, 2, axis=0).astype(np.float32)
        sint = np.repeat(fc[toks, :, 1].T, 2, axis=0).astype(np.float32)
        sint[0::2, :] *= -1.0
        maskp = np.zeros((P, NCH * 64), np.float32)
        for g in range(NCH):
            rows = toks[64 * MG[g]:64 * MG[g] + 64]
            sc = MASK_SRC[g]
            maskp[:, sc:sc + 64] = mask[rows, g * P:(g + 1) * P].T / SCALE
        # clamp the -inf stand-ins into fp16 range (classify() guarantees the
        # mask is pure causal, so any huge-negative value acts as -inf)
        maskp = np.maximum(maskp, MASK_NEG)
        in_maps.append({
            "xt": xt, "wqp": wqp, "wkp": wkp, "wvp": wvp, "wop": wop,
            "cost": np.ascontiguousarray(cost),
            "sint": np.ascontiguousarray(sint),
            "maskp": maskp.astype(NPF16),
            "swapm": swapm, "onesmat": onesmat, "identm": identm,
        })
    return in_maps


_NC_CACHE = {}


def kernel_run(x, start_pos, freqs_cis, mask, wq, wk, wv, wo,
               cfg: Cfg = FULL, trace=False):
    sig = classify(mask, cfg)
    in_maps = make_in_maps(x, freqs_cis, mask, wq, wk, wv, wo, cfg)
    key = (cfg, sig)
    if key not in _NC_CACHE:
        _NC_CACHE[key] = build_nc(cfg)
    nc = _NC_CACHE[key]
    res = run_bass_kernel_spmd(nc, in_maps, core_ids=list(range(8)), trace=trace)
    full = np.empty((2, cfg.S, cfg.D), np.float32)
    for c in range(8):
        b, q = c // 4, c % 4
        toks = local_tokens(q, cfg)
        full[b, toks, :] = res.results[c]["out"]
    return full, res


def kernel(x, start_pos=None, freqs_cis=None, mask=None, wq=None, wk=None,
           wv=None, wo=None):
    full, _ = kernel_run(x, start_pos, freqs_cis, mask, wq, wk, wv, wo)
    return full

